# revision 37
# baseline (speedup 1.0000x reference)
"""Trainium2 Bass kernel for nn_FSSM_Block (S4-style FFT-conv block).

kernel(**inputs) -> (y, h_ratio). Data-parallel over batch: 8 batches -> 8
NeuronCores; kernel generation (Cauchy/Woodbury over L x N) replicated per
core in low-rank (rank-256) form; fc/LN replicated per core on its shard.

All length-4096 DFTs are 2-stage matmul factorizations (64x64 digit split)
with twiddles folded into constant matrices shipped from host; complex
arithmetic via stacked-real matrices; corner turns via PE transposes;
matmul inputs bf16, fp32 PSUM accumulation.
"""
import numpy as np
import ml_dtypes

BSZ, L, H, N = 8, 4096, 256, 64
D1 = 64
NCORES = 8
BF = ml_dtypes.bfloat16

_CACHE = {}


def _host_constants():
    W_L = np.exp(-2j * np.pi / L)
    W64 = np.exp(-2j * np.pi / 64)
    w2L = np.exp(-1j * np.pi / L)
    t1 = np.arange(D1)[:, None]
    j1c = np.arange(D1)[None, :]
    t2 = np.arange(D1)[:, None]

    def stk_in_real(M):        # [t1:64, 128]: real-input stage1 lhsT
        return np.concatenate([M.real, M.imag], axis=1)

    def stk_full(M):           # [(ri,t):2K, (rj,j):2M] complex-stacked lhsT
        return np.block([[M.real, M.imag], [-M.imag, M.real]])

    def stk_real_out(M):       # [(ri,t):2K, M]: real-part-extracting lhsT
        return np.concatenate([M.real, -M.imag], axis=0)

    C = {}
    M1f = W64 ** (t1 * j1c)
    def dup(M):
        return np.concatenate([M, M], axis=0)
    C["S1FE"] = dup(stk_in_real(M1f))
    C["S1FO"] = dup(stk_in_real(M1f * (w2L ** (64 * t1))))
    j2 = np.arange(32)[None, :]
    s2fe = np.empty((D1, 128, 64), np.float32)
    s2fo = np.empty((D1, 128, 64), np.float32)
    for a in range(D1):
        Me = (W_L ** (t2 * a)) * (W64 ** (t2 * j2))
        Mo = (w2L ** t2) * Me
        s2fe[a] = stk_full(Me)
        s2fo[a] = stk_full(Mo)
    C["S2FE"], C["S2FO"] = s2fe, s2fo
    C["S1IF"] = stk_full(W64 ** (-(t1 * j1c)))
    s2if = np.empty((D1, 128, 128), np.float32)
    lhi = np.arange(D1)[None, :]
    for llo in range(D1):
        s2if[llo] = stk_full((1.0 / L) * (W_L ** (-(t2 * llo))) * (W64 ** (-(t2 * lhi))))
    C["S2IF"] = s2if
    j2c = np.arange(32)[:, None]
    tloc = np.arange(D1)[None, :]
    Me1 = 2.0 * (W64 ** (-(j2c * tloc)))
    C["S1IE"] = dup(stk_full(Me1))
    C["S1IO"] = C["S1IE"].copy()
    s2ie = np.empty((D1, 128, 64), np.float32)
    s2io = np.empty((D1, 128, 64), np.float32)
    thi = np.arange(D1)[None, :]
    j1r = np.arange(D1)[:, None]
    for tlo in range(D1):
        Me = (1.0 / (2 * L)) * (W_L ** (-(tlo * j1r))) * (W64 ** (-(j1r * thi)))
        Mo = (np.conj(w2L) ** tlo) * (np.conj(w2L) ** (64 * thi)) * Me
        s2ie[tlo] = stk_real_out(Me)
        s2io[tlo] = stk_real_out(Mo)
    C["S2IE"], C["S2IO"] = s2ie, s2io
    k = np.arange(L)
    Om = np.exp(-2j * np.pi * k / L)
    ratio = (1.0 - Om) / (1.0 + Om)
    cc = 2.0 / (1.0 + Om)
    C["GBASE"] = np.broadcast_to(ratio.imag.astype(np.float32), (D1, L)).copy()
    C["CRE"] = np.broadcast_to(cc.real.astype(np.float32), (D1, L)).copy()
    C["CIM"] = np.broadcast_to(cc.imag.astype(np.float32), (D1, L)).copy()
    C1 = np.zeros((128, 128), np.float32)
    C2 = np.zeros((128, 128), np.float32)
    for chh in range(2):
        for jj in range(32):
            r = chh * 64 + jj
            C1[r, r] = 1.0
            C1[r + 32, r] = -1.0
            C2[r, r + 32] = 1.0
            C2[r + 32, r + 32] = 1.0
    C["CMB1"] = C1
    C["CMB2"] = C2
    C["IDENT"] = np.eye(128, dtype=np.float32)
    C["IDENT2"] = np.concatenate([np.eye(64), np.eye(64)], 0).astype(np.float32)
    C["ONES"] = np.ones((1, 128), np.float32)
    for key in ("S1FE", "S1FO", "S2FE", "S2FO", "S1IF", "S2IF", "S1IE", "S1IO",
                "S2IE", "S2IO", "IDENT", "IDENT2", "ONES", "CMB1", "CMB2"):
        C[key] = C[key].astype(BF)
    return C


def _emit(ctx, tc, io):
    import concourse.bass as bass
    import concourse.mybir as mybir
    from concourse.bass import ts
    nc = tc.nc
    f32, bf16 = mybir.dt.float32, mybir.dt.bfloat16
    AF = mybir.ActivationFunctionType
    OP = mybir.AluOpType
    AX = mybir.AxisListType

    pp = ctx.enter_context(tc.tile_pool(name="persist", bufs=1))
    wp = ctx.enter_context(tc.tile_pool(name="wstream", bufs=4))
    tp = ctx.enter_context(tc.tile_pool(name="temps", bufs=1))
    m2 = ctx.enter_context(tc.tile_pool(name="arena", bufs=8))
    bp = m2
    m1 = m2
    kp = ctx.enter_context(tc.tile_pool(name="kchunk", bufs=1))
    ps = ctx.enter_context(tc.tile_pool(name="psA", bufs=2, space="PSUM"))
    ps2 = ctx.enter_context(tc.tile_pool(name="psB", bufs=2, space="PSUM"))

    ident = pp.tile([128, 128], bf16)
    nc.sync.dma_start(ident[:], io["IDENT"][:])
    ident2 = pp.tile([128, 64], bf16)   # two stacked 64-identities
    nc.sync.dma_start(ident2[:], io["IDENT2"][:])

    def trn(dst_psum, src_sbuf, start=True, stop=True):
        kdim = src_sbuf.shape[0]
        base = src_sbuf.base_partition()
        if kdim == 128:
            idn = ident[:, :]
        else:
            idn = ident2[base: base + 64, :]
        nc.tensor.matmul(dst_psum, src_sbuf, idn, is_transpose=True,
                         start=start, stop=stop)

    _rr = [0]

    def cp(dst, src):
        _rr[0] += 1
        if _rr[0] % 2:
            nc.vector.tensor_copy(dst, src)
        else:
            nc.scalar.copy(dst, src)

    # ---------------- x loads ----------------
    XNB = bp.tile([128, 32 * H], bf16, tag="m2")
    nc.sync.dma_start(XNB[:].rearrange("p (a c) -> p a c", a=32),
                      io["xbf"][:].rearrange("(a p) c -> p a c", p=128))
    XFA = m2.tile([128, 8192], bf16, tag="m2")   # [(cb,t1) x (t2, c128)]
    for cbv in range(2):
        nc.sync.dma_start(
            XFA[cbv * 64:(cbv + 1) * 64, :].rearrange("t1 (t2 c) -> t1 t2 c", t2=64),
            io["xbf"][:].rearrange("(t1 t2) (cb c) -> t1 t2 cb c", t1=64, cb=2)[:, :, cbv, :])
    XCT = m2.tile([128, 2 * L], bf16, tag="m2")  # x^T: [c-blk rows x t]
    for tt in range(32):
        for cb in range(2):
            pt = ps.tile([128, 1024], bf16, tag="ptrn", bufs=2)
            trn(pt[:, 0:128], XNB[:, tt * H + cb * 128: tt * H + cb * 128 + 128])
            cp(XCT[:, cb * L + tt * 128: cb * L + (tt + 1) * 128], pt[:, 0:128])

    # ---------------- scalars ----------------
    stp = pp.tile([1, 1], f32)
    nc.sync.dma_start(stp[:], io["step"][:])
    Dv = pp.tile([1, 1], f32)
    nc.sync.dma_start(Dv[:], io["D"][:])
    rstep = pp.tile([1, 1], f32)
    nc.vector.tensor_scalar_max(rstep[:], stp[:], 1e-6)
    nc.vector.reciprocal(rstep[:], rstep[:])
    nc.scalar.mul(rstep[:], rstep[:], 2.0)
    ones = pp.tile([1, 128], bf16)
    nc.sync.dma_start(ones[:], io["ONES"][:])

    def repl(dst, src_row, n_part, ncols):
        srcb = tp.tile([1, ncols], bf16, tag="replb")
        cp(srcb[:], src_row)
        for q0 in range(0, ncols, 512):
            q1 = min(q0 + 512, ncols)
            pt = ps.tile([n_part, 512], f32, tag="pb512", bufs=2)
            nc.tensor.matmul(pt[:, : q1 - q0], ones[:, :n_part], srcb[:, q0:q1],
                             start=True, stop=True)
            cp(dst[:, q0:q1], pt[:, : q1 - q0])

    lam_re = pp.tile([64, 1], f32)
    nc.sync.dma_start(lam_re[:], io["Lambda_re"][:])
    lam_im = pp.tile([64, 1], f32)
    nc.sync.dma_start(lam_im[:], io["Lambda_im"][:])
    asq = pp.tile([64, 1], f32)
    nc.vector.tensor_tensor(asq[:], lam_re[:], lam_re[:], OP.mult)
    negl = pp.tile([64, 1], f32)
    nc.scalar.mul(negl[:], lam_re[:], -1.0)
    QBP = pp.tile([64, 8], bf16)
    nc.sync.dma_start(QBP[:], io["QBP"][:])

    # ---------------- K side (chunked over k) ----------------
    V2R = m2.tile([128, L], bf16, tag="m2")
    V2I = m2.tile([128, L], bf16, tag="m2")
    rst64 = pp.tile([64, 1], f32)
    repl(rst64[:], rstep[:], 64, 1)
    CK = 256
    for kc in range(L // CK):
        sl = slice(kc * CK, (kc + 1) * CK)
        GBc = kp.tile([64, CK], f32, tag="kc", bufs=12)
        nc.sync.dma_start(GBc[:], io["GBASE"][:][:, sl])
        CREc = kp.tile([64, CK], f32, tag="kc", bufs=12)
        nc.sync.dma_start(CREc[:], io["CRE"][:][:, sl])
        CIMc = kp.tile([64, CK], f32, tag="kc", bufs=12)
        nc.sync.dma_start(CIMc[:], io["CIM"][:][:, sl])
        G = kp.tile([64, CK], f32, tag="kc", bufs=12)
        nc.vector.tensor_scalar_mul(G[:], GBc[:], rst64[:])
        bt = kp.tile([64, CK], f32, tag="kc", bufs=12)
        nc.vector.tensor_scalar(bt[:], G[:], lam_im[:], None, OP.subtract)
        den = kp.tile([64, CK], f32, tag="kc", bufs=12)
        nc.vector.tensor_tensor(den[:], bt[:], bt[:], OP.mult)
        nc.vector.tensor_scalar(den[:], den[:], asq[:], None, OP.add)
        nc.vector.reciprocal(den[:], den[:])
        IVRc = kp.tile([64, CK], f32, tag="kc", bufs=12)
        nc.vector.tensor_scalar_mul(IVRc[:], den[:], negl[:])
        IVIc = kp.tile([64, CK], f32, tag="kc", bufs=12)
        nc.vector.tensor_tensor(IVIc[:], bt[:], den[:], OP.mult)
        nc.scalar.mul(IVIc[:], IVIc[:], -1.0)
        IVRb = kp.tile([64, CK], bf16, tag="kcb", bufs=4)
        cp(IVRb[:], IVRc[:])
        IVIb = kp.tile([64, CK], bf16, tag="kcb", bufs=4)
        cp(IVIb[:], IVIc[:])
        kts = []
        for ci in range(4):
            kk = ps.tile([1, CK], f32, tag="pbsm", bufs=2)
            nc.tensor.matmul(kk[:], QBP[:, ci:ci + 1], IVRb[:], start=True, stop=False)
            nc.tensor.matmul(kk[:], QBP[:, 4 + ci:5 + ci], IVIb[:], start=False, stop=True)
            ktc = kp.tile([1, CK], f32, tag="kc1", bufs=12, name=f"ktc{ci}", uniquify=True)
            cp(ktc[:], kk[:])
            kts.append(ktc)
        KT0, KT1, KT2, KT3 = kts
        onep = kp.tile([1, CK], f32, tag="kc1", bufs=12)
        nc.vector.tensor_scalar(onep[:], KT2[:], 1.0, None, OP.add)
        d2 = kp.tile([1, CK], f32, tag="kc1", bufs=12)
        nc.vector.tensor_tensor(d2[:], onep[:], onep[:], OP.mult)
        t2t = kp.tile([1, CK], f32, tag="kc1", bufs=12)
        nc.vector.tensor_tensor(t2t[:], KT3[:], KT3[:], OP.mult)
        nc.vector.tensor_tensor(d2[:], d2[:], t2t[:], OP.add)
        nc.vector.reciprocal(d2[:], d2[:])
        sre = kp.tile([1, CK], f32, tag="kc1", bufs=12)
        sim = kp.tile([1, CK], f32, tag="kc1", bufs=12)
        ta = kp.tile([1, CK], f32, tag="kc1", bufs=12)
        tb = kp.tile([1, CK], f32, tag="kc1", bufs=12)
        nc.vector.tensor_tensor(ta[:], KT0[:], onep[:], OP.mult)
        nc.vector.tensor_tensor(tb[:], KT1[:], KT3[:], OP.mult)
        nc.vector.tensor_tensor(sre[:], ta[:], tb[:], OP.add)
        nc.vector.tensor_tensor(sre[:], sre[:], d2[:], OP.mult)
        nc.vector.tensor_tensor(ta[:], KT1[:], onep[:], OP.mult)
        nc.vector.tensor_tensor(tb[:], KT0[:], KT3[:], OP.mult)
        nc.vector.tensor_tensor(sim[:], ta[:], tb[:], OP.subtract)
        nc.vector.tensor_tensor(sim[:], sim[:], d2[:], OP.mult)
        SREc = kp.tile([64, CK], f32, tag="kc", bufs=12)
        repl(SREc[:], sre[:], 64, CK)
        SIMc = kp.tile([64, CK], f32, tag="kc", bufs=12)
        repl(SIMc[:], sim[:], 64, CK)
        vt1 = kp.tile([64, CK], f32, tag="kc", bufs=12)
        vt2 = kp.tile([64, CK], f32, tag="kc", bufs=12)
        V1R = kp.tile([64, CK], f32, tag="kc", bufs=12)
        V1I = kp.tile([64, CK], f32, tag="kc", bufs=12)
        nc.vector.tensor_tensor(vt1[:], CREc[:], IVRc[:], OP.mult)
        nc.vector.tensor_tensor(vt2[:], CIMc[:], IVIc[:], OP.mult)
        nc.vector.tensor_tensor(V1R[:], vt1[:], vt2[:], OP.subtract)
        cp(V2R[0:64, sl], V1R[:])
        nc.vector.tensor_tensor(vt1[:], CREc[:], IVIc[:], OP.mult)
        nc.vector.tensor_tensor(vt2[:], CIMc[:], IVRc[:], OP.mult)
        nc.vector.tensor_tensor(V1I[:], vt1[:], vt2[:], OP.add)
        cp(V2I[0:64, sl], V1I[:])
        v2lo = kp.tile([64, CK], bf16, tag="kcb", bufs=4)
        nc.vector.tensor_tensor(vt1[:], SREc[:], V1R[:], OP.mult)
        nc.vector.tensor_tensor(vt2[:], SIMc[:], V1I[:], OP.mult)
        nc.vector.tensor_tensor(vt1[:], vt2[:], vt1[:], OP.subtract)
        cp(v2lo[:], vt1[:])
        nc.sync.dma_start(V2R[64:128, sl], v2lo[:])
        v2lo2 = kp.tile([64, CK], bf16, tag="kcb", bufs=4)
        nc.vector.tensor_tensor(vt1[:], SREc[:], V1I[:], OP.mult)
        nc.vector.tensor_tensor(vt2[:], SIMc[:], V1R[:], OP.mult)
        nc.vector.tensor_tensor(vt1[:], vt1[:], vt2[:], OP.add)
        nc.scalar.mul(vt1[:], vt1[:], -1.0)
        cp(v2lo2[:], vt1[:])
        nc.sync.dma_start(V2I[64:128, sl], v2lo2[:])

    # ---------------- UtT tiles [r:128 x c:256] ----------------
    UTA = pp.tile([128, H], bf16)
    UTB = pp.tile([128, H], bf16)
    PBv = pp.tile([64, 4], f32)
    nc.sync.dma_start(PBv[:], io["PBV"][:])
    for half in range(2):
        cts = []
        for comp in range(2):
            src = io["C_re"] if comp == 0 else io["C_im"]
            cf = tp.tile([128, 64], f32, tag="cf")
            nc.sync.dma_start(cf[:], src[:][half * 128:(half + 1) * 128, :])
            cb_ = tp.tile([128, 64], bf16, tag="cb_")
            cp(cb_[:], cf[:])
            ptc = ps.tile([64, 1024], bf16, tag="ptrn", bufs=2)
            trn(ptc[:, 0:128], cb_[:])
            ctf = tp.tile([64, 128], f32, tag=f"ct{comp}")
            cp(ctf[:], ptc[:, 0:128])
            cts.append(ctf)
        ctre, ctim = cts
        for blk, (wr, wi) in enumerate(((0, 1), (2, 3))):
            tr_ = tp.tile([64, 128], f32, tag="tr_")
            ti_ = tp.tile([64, 128], f32, tag="ti_")
            rr_ = tp.tile([64, 128], bf16, tag="rr_")
            nc.vector.tensor_scalar_mul(tr_[:], ctre[:], PBv[:, wr:wr + 1])
            nc.vector.tensor_scalar_mul(ti_[:], ctim[:], PBv[:, wi:wi + 1])
            nc.vector.tensor_sub(rr_[:], tr_[:], ti_[:])
            if blk == 0:
                cp(UTA[0:64, half * 128:(half + 1) * 128], rr_[:])
            else:
                nc.sync.dma_start(UTA[64:128, half * 128:(half + 1) * 128], rr_[:])
            nc.vector.tensor_scalar_mul(tr_[:], ctim[:], PBv[:, wr:wr + 1])
            nc.vector.tensor_scalar_mul(ti_[:], ctre[:], PBv[:, wi:wi + 1])
            nc.vector.tensor_add(rr_[:], tr_[:], ti_[:])
            nc.scalar.mul(rr_[:], rr_[:], -1.0)
            if blk == 0:
                cp(UTB[0:64, half * 128:(half + 1) * 128], rr_[:])
            else:
                nc.sync.dma_start(UTB[64:128, half * 128:(half + 1) * 128], rr_[:])

    # ---------------- We tiles (even basis, closed form) ----------------
    # cols: [We_re(2048) | We_im(2048)], col j within half; j=2048 bin dropped.
    # WeA rows pair Ut-re block; WeB rows pair Ut-(-im) block:
    #  WeA = We1 = (V[j] + conj V[L-j])/2 ;  WeB = We2 = (V[j] - conj V[L-j])/(2i)
    WEA = m1.tile([128, 2 * 2048], bf16, tag="m2")
    WEB = m1.tile([128, 2 * 2048], bf16, tag="m2")
    w_ = tp.tile([128, 2047], bf16, tag="w_")
    # So A + flip(Brev) needed. flip again unavailable... => flip ONCE via
    # host-shipped permutation through DRAM: dma out to DRAM scratch with
    # negative-stride? DMA APs DO support negative strides (signed steps).
    FLR = m1.tile([128, 2047], bf16, tag="m2")
    FLI = m1.tile([128, 2047], bf16, tag="m2")
    nc.sync.dma_start(FLR[:], V2R[:, 4095:2048:-1])
    nc.sync.dma_start(FLI[:], V2I[:, 4095:2048:-1])
    # FLR[:, i] = V2R[:, 4095-i] = V2R[:, L-(1+i)] = V2R[:, L-j] ✓
    nc.vector.tensor_add(w_[:], V2R[:, 1:2048], FLR[:])
    nc.scalar.activation(WEA[:, 1:2048], w_[:], AF.Copy, scale=0.5)
    nc.vector.tensor_sub(w_[:], V2I[:, 1:2048], FLI[:])
    nc.scalar.activation(WEA[:, 2048 + 1: 2048 + 2048], w_[:], AF.Copy, scale=0.5)
    nc.vector.tensor_add(w_[:], V2I[:, 1:2048], FLI[:])
    nc.scalar.activation(WEB[:, 1:2048], w_[:], AF.Copy, scale=0.5)
    nc.vector.tensor_sub(w_[:], V2R[:, 1:2048], FLR[:])
    nc.scalar.activation(WEB[:, 2048 + 1: 2048 + 2048], w_[:], AF.Copy, scale=-0.5)
    # j=0 columns: We1[0]=Re(V[0]); We1_im[0]=0... (V + conj V)/2 = Re(V):
    nc.scalar.activation(WEA[:, 0:1], V2R[:, 0:1], AF.Copy)
    nc.gpsimd.memset(WEA[:, 2048:2049], 0.0)
    # We2[0] = (V - conj V)/2i = i*Im(V)/i = Im(V): re part = Im(V), im part = 0
    nc.scalar.activation(WEB[:, 0:1], V2I[:, 0:1], AF.Copy)
    nc.gpsimd.memset(WEB[:, 2048:2049], 0.0)

    # ---------------- KFE mix ----------------
    KFE = m2.tile([128, 8192], bf16, tag="m2")     # [(chalf, rk*32+j2) x (j1, c128)]
    WEAr = WEA[:].rearrange("p (rk j2 j1) -> p rk j2 j1", rk=2, j2=32, j1=64)
    WEBr = WEB[:].rearrange("p (rk j2 j1) -> p rk j2 j1", rk=2, j2=32, j1=64)
    for j1 in range(64):
        pk = ps.tile([128, 128], f32, tag="pbsm", bufs=2)
        lhsA = WEAr[:, :, :, j1].rearrange("p rk j2 -> p (rk j2)")
        lhsB = WEBr[:, :, :, j1].rearrange("p rk j2 -> p (rk j2)")
        for ch in range(2):
            outp = pk[ch * 64: ch * 64 + 64, :]
            nc.tensor.matmul(outp, lhsA, UTA[:, ch * 128:(ch + 1) * 128], start=True, stop=False)
            nc.tensor.matmul(outp, lhsB, UTB[:, ch * 128:(ch + 1) * 128], start=False, stop=True)
        cp(KFE[:, j1 * 128:(j1 + 1) * 128], pk[:])

    # ---------------- V stacked + ifft + odd basis ----------------
    VSTK = m2.tile([128, 8192], bf16, tag="m2")    # [(ri,k1):128 x (n':128, k2:64)]
    for plane_i, plane in ((0, V2R), (1, V2I)):
        pr = plane[:].rearrange("n (k1 k2) -> n k1 k2", k1=64)
        dst = VSTK[plane_i * 64: plane_i * 64 + 64, :].rearrange("k1 (n k2) -> k1 n k2", n=128)
        for nh in range(2):
            for k2v in range(64):
                ptv = ps.tile([128, 1024], bf16, tag="ptrn", bufs=2)
                trn(ptv[plane_i * 64: plane_i * 64 + 64, 0:64], pr[nh * 64:(nh + 1) * 64, :, k2v])
                cp(dst[:, nh * 64:(nh + 1) * 64, k2v],
                   ptv[plane_i * 64: plane_i * 64 + 64, 0:64])

    SW = {k: io[k] for k in ("S2FE", "S2FO", "S2IF", "S2IE", "S2IO")}
    S1T = {}
    for k in ("S1FE", "S1FO", "S1IF", "S1IE", "S1IO"):
        t_ = pp.tile([io[k].shape[0], io[k].shape[1]], bf16, tag=f"s1_{k}")
        nc.sync.dma_start(t_[:], io[k][:])
        S1T[k] = t_

    def s2w(name, j1):
        w = wp.tile([128, SW[name].shape[2]], bf16, tag=f"w_{name}")
        nc.sync.dma_start(w[:], SW[name][j1])
        return w

    VT = m2.tile([128, 8192], bf16, tag="m2")      # [(rk,l_hi) x (n':128, l_lo:64)]
    s1k = bp.tile([128, 8192], bf16, tag="m2")
    for q in range(16):
        p1 = ps.tile([128, 512], f32, tag="pb512", bufs=2)
        nc.tensor.matmul(p1[:], S1T["S1IF"][:], VSTK[:, ts(q, 512)], start=True, stop=True)
        cp(s1k[:, ts(q, 512)], p1[:])
    tko = bp.tile([128, 8192], bf16, tag="m2")
    s1kr = s1k[:].rearrange("p (n k2) -> p n k2", n=128)
    tkor = tko[:].rearrange("p (n llo) -> p n llo", n=128)
    for n_ in range(128):
        ptA = ps.tile([128, 1024], bf16, tag="ptrn", bufs=2)
        ptB = ps.tile([128, 1024], bf16, tag="ptrn", bufs=2)
        trn(ptA[0:64, 0:64], s1kr[0:64, n_, :])
        trn(ptB[64:128, 0:64], s1kr[64:128, n_, :])
        cp(tkor[0:64, n_, :], ptA[0:64, 0:64])
        cp(tkor[64:128, n_, :], ptB[64:128, 0:64])
    for llo in range(64):
        w = s2w("S2IF", llo)
        p2 = ps2.tile([128, 128], f32, tag="pb2", bufs=2)
        nc.tensor.matmul(p2[:], w[:], tkor[:, :, llo], start=True, stop=True)
        cp(VT[:].rearrange("p (n llo) -> p n llo", n=128)[:, :, llo], p2[:])

    WO = m2.tile([128, 2 * 4096], bf16, tag="m2")  # [modes:128 x (plane, j1, rk*32+j2)]
    WOr = WO[:].rearrange("p (h j1 m) -> p h j1 m", h=2, j1=64)
    for plane in range(2):
        s1p_ = bp.tile([128, 8192], bf16, tag="m2")
        for q in range(16):
            p1 = ps.tile([128, 512], f32, tag="pb512", bufs=2)
            nc.tensor.matmul(p1[:], S1T["S1FO"][plane * 64:(plane + 1) * 64, :],
                             VT[plane * 64:(plane + 1) * 64, ts(q, 512)], start=True, stop=True)
            cp(s1p_[:, ts(q, 512)], p1[:])
        tk2 = bp.tile([128, 8192], bf16, tag="m2")
        s1pr = s1p_[:].rearrange("p (n llo) -> p n llo", n=128)
        tk2r = tk2[:].rearrange("p (n j1) -> p n j1", n=128)
        for n_ in range(128):
            ptA = ps.tile([128, 1024], bf16, tag="ptrn", bufs=2)
            ptB = ps.tile([128, 1024], bf16, tag="ptrn", bufs=2)
            trn(ptA[0:64, 0:64], s1pr[0:64, n_, :])
            trn(ptB[64:128, 0:64], s1pr[64:128, n_, :])
            cp(tk2r[0:64, n_, :], ptA[0:64, 0:64])
            cp(tk2r[64:128, n_, :], ptB[64:128, 0:64])
        for j1 in range(64):
            w = s2w("S2FO", j1)
            pko = ps2.tile([128, 64], f32, tag="pb2", bufs=2)
            nc.tensor.matmul(pko[:], tk2r[:, :, j1], w[:], start=True, stop=True)
            cp(WOr[:, plane, j1, :], pko[:])

    KFO = m2.tile([128, 8192], bf16, tag="m2")
    for j1 in range(64):
        pk = ps.tile([128, 128], f32, tag="pbsm", bufs=2)
        for ch in range(2):
            outp = pk[ch * 64: ch * 64 + 64, :]
            nc.tensor.matmul(outp, WOr[:, 0, j1, :], UTA[:, ch * 128:(ch + 1) * 128], start=True, stop=False)
            nc.tensor.matmul(outp, WOr[:, 1, j1, :], UTB[:, ch * 128:(ch + 1) * 128], start=False, stop=True)
        cp(KFO[:, j1 * 128:(j1 + 1) * 128], pk[:])

    # ---------------- forward transforms of x ----------------
    SPE = m2.tile([128, 8192], bf16, tag="m2")     # [(chalf, rk*32+j2) x (j1, c128)]
    SPO = m2.tile([128, 8192], bf16, tag="m2")

    def fwd(n1, n2, SPdst):
        for ch in range(2):
            XFr = XFA[ch * 64:(ch + 1) * 64, :].rearrange("p (t2 c) -> p t2 c", t2=64)
            s1o = bp.tile([128, 8192], bf16, tag="m2")
            for q in range(16):
                p1 = ps.tile([128, 512], f32, tag="pb512", bufs=2)
                rhs = XFr[:, q * 4:(q + 1) * 4, :]
                nc.tensor.matmul(p1[:], S1T[n1][ch * 64:(ch + 1) * 64, :], rhs, start=True, stop=True)
                cp(s1o[:, ts(q, 512)], p1[:])
            turned = bp.tile([128, 8192], bf16, tag="m2")
            s1or = s1o[:].rearrange("p (t2 c) -> p t2 c", t2=64)
            turnr = turned[:].rearrange("p (j1 c) -> p j1 c", j1=64)
            for cc_ in range(128):
                ptA = ps.tile([128, 1024], bf16, tag="ptrn", bufs=2)
                ptB = ps.tile([128, 1024], bf16, tag="ptrn", bufs=2)
                trn(ptA[0:64, 0:64], s1or[0:64, :, cc_])
                trn(ptB[64:128, 0:64], s1or[64:128, :, cc_])
                cp(turnr[0:64, :, cc_], ptA[0:64, 0:64])
                cp(turnr[64:128, :, cc_], ptB[64:128, 0:64])
            # wait: turnr slice is [128 x 64] (j1 axis): ptt is [(rj,t2) x j1?]:
            # trn gave [t2 x j1] per rj -> rows (rj,t2):128, cols j1:64 ✓
            for j1 in range(64):
                w = s2w(n2, j1)
                p2 = ps2.tile([64, 128], f32, tag="pb2", bufs=2)
                nc.tensor.matmul(p2[:], w[:], turnr[:, j1, :].rearrange("p c -> p c") if False else turned[:].rearrange("p (j1 c) -> p j1 c", j1=64)[:, j1, :], start=True, stop=True)
                if ch == 0:
                    cp(SPdst[0:64, :].rearrange("p (j1 c) -> p j1 c", j1=64)[:, j1, :], p2[:])
                else:
                    stmp = tp.tile([64, 128], bf16, tag="stmp")
                    cp(stmp[:], p2[:])
                    nc.sync.dma_start(SPdst[64:128, :].rearrange("p (j1 c) -> p j1 c", j1=64)[:, j1, :], stmp[:])

    # NOTE turn layout check: s1 out rows = (rj, j1)?? lhsT S1FE [64 x 128]:
    # M-cols = (rj*64 + j1) -> out rows 0:64 = re(j1), 64:128 = im(j1) ✓
    # turn in: [j1:64 x t2-cols] per rj; out [t2... wait trn in = s1or[rj-half, :, cc]
    # = [j1? NO rows of s1or are (rj,j1): slice rj-half rows = j1:64, dims [j1 x t2]
    # Hmm s1or[p, t2, c]: p=(rj,j1): s1or[rj*64:(rj+1)*64, :, cc_] = [j1:64, t2:64]
    # -> transpose -> [t2 x j1] written at rows rj*64 (=(rj,t2)) cols j1 ✓✓
    fwd("S1FE", "S2FE", SPE)
    fwd("S1FO", "S2FO", SPO)

    # ---------------- products (via swap + PE combine) ----------------
    CMB1 = pp.tile([128, 128], bf16)
    nc.sync.dma_start(CMB1[:], io["CMB1"][:])
    CMB2 = pp.tile([128, 128], bf16)
    nc.sync.dma_start(CMB2[:], io["CMB2"][:])

    def swap_rows(KF):
        KFs = m2.tile([128, 8192], bf16, tag="m2", name="kfs")
        for chh in range(2):
            nc.sync.dma_start(KFs[chh * 64: chh * 64 + 32, :], KF[chh * 64 + 32: chh * 64 + 64, :])
            nc.sync.dma_start(KFs[chh * 64 + 32: chh * 64 + 64, :], KF[chh * 64: chh * 64 + 32, :])
        return KFs

    def cprod(SP, KF):
        KFs = swap_rows(KF)
        for q in range(16):
            T1 = tp.tile([128, 512], bf16, tag="pt1")
            T2 = tp.tile([128, 512], bf16, tag="pt2")
            nc.vector.tensor_tensor(T1[:], SP[:, ts(q, 512)], KF[:, ts(q, 512)], OP.mult)
            nc.gpsimd.tensor_tensor(T2[:], SP[:, ts(q, 512)], KFs[:, ts(q, 512)], OP.mult)
            pq = ps.tile([128, 512], f32, tag="pb512", bufs=2)
            nc.tensor.matmul(pq[:], CMB1[:], T1[:], start=True, stop=False)
            nc.tensor.matmul(pq[:], CMB2[:], T2[:], start=False, stop=True)
            cp(SP[:, ts(q, 512)], pq[:])

    for ch in range(2):
        for rk in range(2):
            r0 = SPE[ch * 64 + rk * 32: ch * 64 + rk * 32 + 1, 0:128]
            nc.scalar.mul(r0, r0, 0.5)
    cprod(SPE, KFE)
    cprod(SPO, KFO)

    # ---------------- h_ratio |x| & x*D ----------------
    AXm = pp.tile([128, 2], f32)
    for cb in range(2):
        nc.vector.tensor_reduce(AXm[:, cb:cb + 1], XCT[:, cb * L:(cb + 1) * L],
                                AX.X, OP.max, apply_absolute_value=True)
    Dv128 = pp.tile([128, 1], f32)
    repl(Dv128[:], Dv[:], 128, 1)
    nc.vector.tensor_scalar_mul(XCT[:], XCT[:], Dv128[:])   # XCT becomes x*D

    # ---------------- inverse + fused gelu1 ----------------
    Y1CT = m2.tile([128, 2 * L], bf16, tag="m2")
    SPEr = SPE[:].rearrange("p (j1 c) -> p j1 c", j1=64)
    SPOr = SPO[:].rearrange("p (j1 c) -> p j1 c", j1=64)
    for ch in range(2):
        e1 = bp.tile([128, 8192], bf16, tag="m2")
        o1 = bp.tile([128, 8192], bf16, tag="m2")
        for (S1n, SPr, dst) in (("S1IE", SPEr, e1), ("S1IO", SPOr, o1)):
            for q in range(16):
                p1 = ps.tile([128, 512], f32, tag="pb512", bufs=2)
                rhs = SPr[ch * 64:(ch + 1) * 64, q * 4:(q + 1) * 4, :]
                nc.tensor.matmul(p1[:], S1T[S1n][ch * 64:(ch + 1) * 64, :], rhs, start=True, stop=True)
                cp(dst[:, ts(q, 512)], p1[:])
        te = bp.tile([128, 8192], bf16, tag="m2")
        to_ = bp.tile([128, 8192], bf16, tag="m2")
        for (src, dstt) in ((e1, te), (o1, to_)):
            srcr = src[:].rearrange("p (j1 c) -> p j1 c", j1=64)
            dstr = dstt[:].rearrange("p (c tlo) -> p c tlo", c=128)
            for cc_ in range(128):
                ptA = ps.tile([128, 1024], bf16, tag="ptrn", bufs=2)
                ptB = ps.tile([128, 1024], bf16, tag="ptrn", bufs=2)
                trn(ptA[0:64, 0:64], srcr[0:64, :, cc_])
                trn(ptB[64:128, 0:64], srcr[64:128, :, cc_])
                cp(dstr[0:64, cc_, :], ptA[0:64, 0:64])
                cp(dstr[64:128, cc_, :], ptB[64:128, 0:64])
        ter = te[:].rearrange("p (c tlo) -> p c tlo", c=128)
        tor = to_[:].rearrange("p (c tlo) -> p c tlo", c=128)
        for tlo in range(64):
            we_ = s2w("S2IE", tlo)
            wo_ = s2w("S2IO", tlo)
            pi = ps2.tile([128, 64], f32, tag="pb2", bufs=2)
            nc.tensor.matmul(pi[:], ter[:, :, tlo], we_[:], start=True, stop=False)
            nc.tensor.matmul(pi[:], tor[:, :, tlo], wo_[:], start=False, stop=True)
            hx = tp.tile([128, 64], f32, tag="hx")
            xsl = XCT[:].rearrange("p (cb thi tlo) -> p cb thi tlo", cb=2, thi=64)[:, ch, :, tlo]
            nc.vector.tensor_tensor(hx[:], pi[:], xsl, OP.add)
            ysl = Y1CT[:].rearrange("p (cb thi tlo) -> p cb thi tlo", cb=2, thi=64)[:, ch, :, tlo]
            nc.scalar.activation(ysl, hx[:], AF.Gelu)

    # ---------------- fc + tail ----------------
    WFT0 = pp.tile([128, H], bf16)
    WFT1 = pp.tile([128, H], bf16)
    for ob in range(2):
        wf = tp.tile([128, H], f32, tag="wf")
        nc.sync.dma_start(wf[:], io["W_fc"][:][ob * 128:(ob + 1) * 128, :])
        wfb = tp.tile([128, H], bf16, tag="wfb")
        cp(wfb[:], wf[:])
        for cb in range(2):
            ptw = ps.tile([128, 1024], bf16, tag="ptrn", bufs=2)
            trn(ptw[:, 0:128], wfb[:, cb * 128:(cb + 1) * 128])
            dstW = WFT0 if cb == 0 else WFT1
            cp(dstW[:, ob * 128:(ob + 1) * 128], ptw[:, 0:128])
    BFC = pp.tile([128, H], f32)
    b1 = tp.tile([1, H], f32, tag="b1")
    nc.sync.dma_start(b1[:], io["b_fc"][:])
    repl(BFC[:], b1[:], 128, H)
    LNG = pp.tile([128, H], f32)
    g1 = tp.tile([1, H], f32, tag="g1")
    nc.sync.dma_start(g1[:], io["ln_g"][:])
    repl(LNG[:], g1[:], 128, H)
    LNB = pp.tile([128, H], f32)
    bb1 = tp.tile([1, H], f32, tag="bb1")
    nc.sync.dma_start(bb1[:], io["ln_b"][:])
    repl(LNB[:], bb1[:], 128, H)

    MYC = pp.tile([128, H], f32)
    nc.gpsimd.memset(MYC[:], 0.0)
    yout = io["y"][:].rearrange("(a p) c -> p a c", p=128)
    for tt in range(32):
        pf = ps.tile([128, H], f32, tag="pbsm", bufs=2)
        nc.tensor.matmul(pf[:], Y1CT[:, tt * 128:(tt + 1) * 128], WFT0[:], start=True, stop=False)
        nc.tensor.matmul(pf[:], Y1CT[:, L + tt * 128: L + (tt + 1) * 128], WFT1[:], start=False, stop=True)
        y2 = tp.tile([128, H], f32, tag="y2")
        nc.vector.tensor_tensor(y2[:], pf[:], BFC[:], OP.add)
        nc.scalar.activation(y2[:], y2[:], AF.Gelu)
        xct_ = tp.tile([128, H], f32, tag="xt_")
        nc.sync.dma_start(xct_[:], io["x"][:].rearrange("(a p) c -> p a c", p=128)[:, tt, :])
        nc.vector.tensor_tensor(y2[:], y2[:], xct_[:], OP.add)
        mu = tp.tile([128, 1], f32, tag="mu")
        nc.vector.tensor_reduce(mu[:], y2[:], AX.X, OP.add)
        nc.scalar.mul(mu[:], mu[:], 1.0 / H)
        nc.vector.tensor_scalar(y2[:], y2[:], mu[:], None, OP.subtract)
        sq = tp.tile([128, H], f32, tag="sq")
        nc.vector.tensor_tensor(sq[:], y2[:], y2[:], OP.mult)
        var = tp.tile([128, 1], f32, tag="var")
        nc.vector.tensor_reduce(var[:], sq[:], AX.X, OP.add)
        nc.vector.tensor_scalar(var[:], var[:], 1.0 / H, 1e-5, OP.mult, OP.add)
        rstd = tp.tile([128, 1], f32, tag="rstd")
        nc.vector.reciprocal(rstd[:], var[:])
        nc.scalar.activation(rstd[:], rstd[:], AF.Sqrt)
        nc.vector.tensor_scalar_mul(y2[:], y2[:], rstd[:])
        nc.vector.tensor_tensor(y2[:], y2[:], LNG[:], OP.mult)
        nc.vector.tensor_tensor(y2[:], y2[:], LNB[:], OP.add)
        nc.sync.dma_start(yout[:, tt, :], y2[:])
        ay = tp.tile([128, H], f32, tag="ay")
        nc.scalar.activation(ay[:], y2[:], AF.Abs)
        nc.vector.tensor_tensor(MYC[:], MYC[:], ay[:], OP.max)

    HRT = pp.tile([128, 2], f32)
    for cb in range(2):
        mycb = tp.tile([128, 128], bf16, tag="mycb")
        cp(mycb[:], MYC[:, cb * 128:(cb + 1) * 128])
        pt = ps.tile([128, 1024], bf16, tag="ptrn", bufs=2)
        trn(pt[:, 0:128], mycb[:])
        mt = tp.tile([128, 128], f32, tag="mt")
        cp(mt[:], pt[:, 0:128])
        nc.vector.tensor_reduce(HRT[:, cb:cb + 1], mt[:], AX.X, OP.max)
    nc.vector.tensor_scalar(AXm[:], AXm[:], 1e-6, None, OP.add)
    RAX = pp.tile([128, 2], f32)
    nc.vector.reciprocal(RAX[:], AXm[:])
    HR = pp.tile([128, 2], f32)
    nc.vector.tensor_tensor(HR[:], HRT[:], RAX[:], OP.mult)
    nc.sync.dma_start(io["hr"][:].rearrange("(b c) -> c b", c=128), HR[:])


def _build_program():
    from contextlib import ExitStack
    import concourse.tile as tile
    from concourse import bacc, mybir

    consts = _CACHE["consts"]
    nc = bacc.Bacc("TRN2", target_bir_lowering=False, debug=False)
    io = {}
    f32, bf16 = mybir.dt.float32, mybir.dt.bfloat16

    def din(name, shape, dt):
        io[name] = nc.dram_tensor(name, list(shape), dt, kind="ExternalInput").ap()

    din("x", (L, H), f32)
    din("xbf", (L, H), bf16)
    din("Lambda_re", (N,), f32)
    din("Lambda_im", (N,), f32)
    din("step", (1,), f32)
    din("D", (1,), f32)
    din("b_fc", (H,), f32)
    din("ln_g", (H,), f32)
    din("ln_b", (H,), f32)
    din("C_re", (H, N), f32)
    din("C_im", (H, N), f32)
    din("W_fc", (H, H), f32)
    din("QBP", (N, 8), bf16)
    din("PBV", (N, 4), f32)
    din("REV", (128, 128), bf16)
    for k, v in consts.items():
        din(k, v.shape, bf16 if v.dtype == BF else f32)
    io["y"] = nc.dram_tensor("y", [L, H], f32, kind="ExternalOutput").ap()
    io["hr"] = nc.dram_tensor("hr", [H], f32, kind="ExternalOutput").ap()

    with tile.TileContext(nc) as tc:
        with ExitStack() as ctx:
            _emit(ctx, tc, io)
    nc.compile()
    return nc


def _prep_inputs(inputs):
    consts = _CACHE["consts"]
    base = {k: np.ascontiguousarray(v) for k, v in consts.items()}
    Qc = np.asarray(inputs["Q_re"]) + 1j * np.asarray(inputs["Q_im"])
    Bc = np.asarray(inputs["B_re"]) + 1j * np.asarray(inputs["B_im"])
    Pc = np.asarray(inputs["P_re"]) + 1j * np.asarray(inputs["P_im"])
    QB, QP = Qc * Bc, Qc * Pc
    base["QBP"] = np.stack([QB.real, QB.imag, QP.real, QP.imag,
                            -QB.imag, QB.real, -QP.imag, QP.real], 1).astype(BF)
    base["PBV"] = np.stack([Bc.real, Bc.imag, Pc.real, Pc.imag], 1).astype(np.float32)
    base["REV"] = np.eye(128, dtype=np.float32)[::-1].copy().astype(BF)
    for nm in ("Lambda_re", "Lambda_im", "step", "D", "b_fc", "ln_g", "ln_b",
               "C_re", "C_im", "W_fc"):
        base[nm] = np.ascontiguousarray(np.asarray(inputs[nm], np.float32))
    maps = []
    for b in range(BSZ):
        m = dict(base)
        xb = np.ascontiguousarray(np.asarray(inputs["x"][b], np.float32))
        m["x"] = xb
        m["xbf"] = xb.astype(BF)
        maps.append(m)
    return maps


def kernel(**inputs):
    if "nc" not in _CACHE:
        _CACHE["consts"] = _host_constants()
        _CACHE["nc"] = _build_program()
    nc = _CACHE["nc"]
    in_maps = _prep_inputs(inputs)
    from concourse.bass_utils import run_bass_kernel_spmd
    res = run_bass_kernel_spmd(nc, in_maps, list(range(NCORES)))
    y = np.stack([res.results[b]["y"] for b in range(BSZ)], 0).astype(np.float32)
    hr = np.stack([res.results[b]["hr"] for b in range(BSZ)], 0).astype(np.float32)
    return y, hr
